# revision 1
# baseline (speedup 1.0000x reference)
"""Boundary-map kernel for Trainium2 (Bass/Tile), 8-core SPMD.

Math: a pixel is an edge pixel iff its radius-2 Euclidean disk (clipped to the
zero-padded array) contains both a 1 and a 0 of some class's one-hot map.
Equivalently (disk is 4-connected): there exists a 4-adjacent pair of pixels
inside the disk with different labels, OR the disk is uniform-nonzero and
touches the pad ring.  With label maps zero-padded by 2, let
    DH(i,j) = [x(i,j) != x(i,j+1)],   DV(i,j) = |x(i+1,j) - x(i,j)|
and dilate each by the set of in-disk pair positions:
    SH = {(0,-2),(0,-1),(0,0),(0,1),(+-1,-1),(+-1,0)}
    SV = {(-2,0),(-1,0),(0,0),(1,0),(-1,+-1),(0,+-1)}
    edge = (sum_{s in SH} DH(p+s) + sum_{s in SV} DV(p+s)) > 0
The zero pad makes the pad-adjacent DV terms fire exactly when the reference's
border term (disk touches pad AND has a nonzero) fires, so no explicit border
handling is needed (verified exhaustively against the reference in numpy).

Layout: row-major tiles [128 partitions = rows, free = cols].  Horizontal
(column) shifts are free-dim AP offsets on DVE; vertical (row) shifts are
band-matrix matmuls on the TensorEngine accumulating into PSUM, with all
row-tap weights folded into five constant 128x128 bf16 band matrices.
Each core: two 128-row input tiles (124 output rows each => 248 rows/core,
8 cores x 248 = 1984 rows) plus one 36-row x 516-col strip tile covering a
1/4-width slice of the last 32 rows of the batch (2 batches x 4 col-slices).
"""

import numpy as np
import ml_dtypes
from contextlib import ExitStack

import concourse.bass as bass
import concourse.bacc as bacc
import concourse.mybir as mybir
import concourse.tile as tile
from concourse import bass_utils

BF16 = mybir.dt.bfloat16
F32 = mybir.dt.float32
I32 = mybir.dt.int32
I8 = mybir.dt.int8
OP = mybir.AluOpType
AF = mybir.ActivationFunctionType

B, H, W = 2, 1024, 2048
RPC = 248            # rows per core from full-width tiles (2 tiles x 124)
SR, SC = 32, 512     # strip rows / cols per core
XROWS, XCOLS = RPC + 4, W + 4        # 252 x 2052 per-core input (2-halo each side)
SXROWS, SXCOLS = SR + 4, SC + 4      # 36 x 516 strip input
NCORES = 8
CHUNK = 512          # PSUM bank width in fp32

PROFILE = False
LAST_EXEC_NS = None
LAST_RESULTS = None

WNAMES = ("w_dv", "w_11", "w_i", "w_v4", "w_v2")


def _band(taps, P=128):
    w = np.zeros((P, P), np.float32)  # [k, m]: out row m sums w[k,m]*src[k]
    for m in range(P):
        for t, v in taps:
            k = m + t
            if 0 <= k < P:
                w[k, m] += v
    return w.astype(ml_dtypes.bfloat16)


def make_weights():
    wd = {
        "w_dv": _band([(0, -1.0), (1, 1.0)]),                       # DV(m) = x(m+1)-x(m)
        "w_11": _band([(-1, 1.0), (1, 1.0)]),                       # taps m-1, m+1
        "w_i": _band([(0, 1.0)]),                                   # identity
        "w_v4": _band([(-2, 1.0), (-1, 1.0), (0, 1.0), (1, 1.0)]),  # taps m-2..m+1
        "w_v2": _band([(-1, 1.0), (0, 1.0)]),                       # taps m-1, m
    }
    # single concatenated tensor -> single DMA (keeps total HWDGE DMA count <= 8
    # so Tile never reuses a DMA-HW proc, which would add a second sync wait that
    # walrus' PSEUDO_DMA_DIRECT2D lowering cannot encode)
    return np.concatenate([wd[k] for k in WNAMES], axis=1)


def _job(nc, ctx, sb, ps, wt, src, r0, P, C, dst, yr0, V, O):
    """Process one tile: src rows [r0, r0+P) x cols [0, C); emit dst rows
    [yr0, yr0+V) x cols [0, O).  Tile row p corresponds to output row
    yr0 + p - 2; tile col j corresponds to output col j - 2."""
    xi = sb.tile([P, C], I32, tag="xi")
    nc.sync.dma_start(xi[:, :], src[r0:r0 + P, :])

    xb = sb.tile([P, C], BF16, tag="xb")
    nc.scalar.activation(out=xb[:, :], in_=xi[:, :], func=AF.Copy)

    # DH(p, j) = [x(p,j) != x(p,j+1)], valid j in [0, C-1)
    DH = sb.tile([P, C], BF16, tag="dh")
    nc.vector.tensor_tensor(out=DH[:, 0:C - 1], in0=xb[:, 0:C - 1],
                            in1=xb[:, 1:C], op=OP.not_equal)
    # H2(j) = DH(j-1) + DH(j), valid j in [1, C-1)
    H2 = sb.tile([P, C], BF16, tag="h2")
    nc.vector.tensor_tensor(out=H2[:, 1:C - 1], in0=DH[:, 0:C - 2],
                            in1=DH[:, 1:C - 1], op=OP.add)
    # H4p(j) = H4(j+1) = H2(j) + H2(j+2), valid j in [1, C-3)
    H4p = sb.tile([P, C], BF16, tag="h4")
    nc.vector.tensor_tensor(out=H4p[:, 1:C - 3], in0=H2[:, 1:C - 3],
                            in1=H2[:, 3:C - 1], op=OP.add)

    # DVa(m, j) = |x(m+1,j) - x(m,j)| via PE band matmul + ACT abs
    DVa = sb.tile([128, C], BF16, tag="dva")
    for c0 in range(0, C, CHUNK):
        n = min(CHUNK, C - c0)
        pdv = ps.tile([128, CHUNK], F32, tag="pdv")
        nc.tensor.matmul(out=pdv[:, :n], lhsT=wt["w_dv"][0:P, :],
                         rhs=xb[:, c0:c0 + n], start=True, stop=True)
        nc.scalar.activation(out=DVa[:, c0:c0 + n], in_=pdv[:, :n], func=AF.Abs)

    # DVHp(j) = DVH(j+1) = DVa(j) + DVa(j+2), valid j in [0, C-2)
    DVHp = sb.tile([128, C], BF16, tag="dvh")
    nc.vector.tensor_tensor(out=DVHp[:, 0:C - 2], in0=DVa[:, 0:C - 2],
                            in1=DVa[:, 2:C], op=OP.add)

    # NU accumulation + threshold, per 512-col chunk of the output range
    e1 = sb.tile([128, C], I8, tag="e1")
    for j0 in range(2, 2 + O, CHUNK):
        n = min(CHUNK, 2 + O - j0)
        pnu = ps.tile([128, CHUNK], F32, tag="pnu")
        nc.tensor.matmul(out=pnu[:, :n], lhsT=wt["w_11"][0:P, :],
                         rhs=H2[0:P, j0:j0 + n], start=True, stop=False)
        nc.tensor.matmul(out=pnu[:, :n], lhsT=wt["w_i"][0:P, :],
                         rhs=H4p[0:P, j0 - 1:j0 - 1 + n], start=False, stop=False)
        nc.tensor.matmul(out=pnu[:, :n], lhsT=wt["w_v4"][0:128, :],
                         rhs=DVa[0:128, j0:j0 + n], start=False, stop=False)
        nc.tensor.matmul(out=pnu[:, :n], lhsT=wt["w_v2"][0:128, :],
                         rhs=DVHp[0:128, j0 - 1:j0 - 1 + n], start=False, stop=True)
        nc.vector.tensor_scalar(out=e1[:, j0:j0 + n], in0=pnu[:, :n],
                                scalar1=0.0, scalar2=None, op0=OP.is_gt)

    nc.sync.dma_start(dst[yr0:yr0 + V, :], e1[2:2 + V, 2:2 + O])


def build_nc():
    # Bacc (not raw Bass): its compile() runs generate_event_semaphores(),
    # which legalizes multi-wait instructions (the TileContext tail drain
    # carries one wait per engine + DMA proc — more than walrus' TPB_CTRL
    # lowering accepts) into event-semaphore chains.
    nc = bacc.Bacc("TRN2", target_bir_lowering=False, debug=False)
    x = nc.dram_tensor("x", [XROWS, XCOLS], I32, kind="ExternalInput").ap()
    xs = nc.dram_tensor("xs", [SXROWS, SXCOLS], I32, kind="ExternalInput").ap()
    wcat = nc.dram_tensor("wcat", [128, 128 * len(WNAMES)], BF16,
                          kind="ExternalInput").ap()
    y = nc.dram_tensor("y", [RPC, W], I8, kind="ExternalOutput").ap()
    ys = nc.dram_tensor("ys", [SR, SC], I8, kind="ExternalOutput").ap()

    with ExitStack() as ctx:
        tc = ctx.enter_context(tile.TileContext(nc))
        wp = ctx.enter_context(tc.tile_pool(name="wp", bufs=1))
        sb = ctx.enter_context(tc.tile_pool(name="sb", bufs=3))
        ps = ctx.enter_context(tc.tile_pool(name="ps", bufs=2, space="PSUM"))
        wtile = wp.tile([128, 128 * len(WNAMES)], BF16, name="wtile")
        nc.sync.dma_start(wtile[:, :], wcat)
        wt = {k: wtile[:, 128 * i:128 * (i + 1)] for i, k in enumerate(WNAMES)}
        _job(nc, ctx, sb, ps, wt, x, 0, 128, XCOLS, y, 0, 124, W)
        _job(nc, ctx, sb, ps, wt, x, 124, 128, XCOLS, y, 124, 124, W)
        _job(nc, ctx, sb, ps, wt, xs, 0, SXROWS, SXCOLS, ys, 0, SR, SC)
    nc.compile()
    return nc


def make_in_maps(gtmasks):
    lab = np.asarray(gtmasks)[:, 0]  # (B, H, W) int32
    wcat = make_weights()
    padded = [np.pad(lab[b], ((2, 2), (2, 2))) for b in range(B)]
    in_maps = []
    for c in range(NCORES):
        b, q = divmod(c, B * 2)  # 4 cores per batch
        xf = padded[b]
        im = {
            "x": np.ascontiguousarray(xf[RPC * q: RPC * q + XROWS, :]),
            "xs": np.ascontiguousarray(xf[H - SR + 2 - 2: H - SR + 2 - 2 + SXROWS,
                                          SC * q: SC * q + SXCOLS]),
        }
        im["wcat"] = wcat
        in_maps.append(im)
    return in_maps


def assemble(results):
    out = np.zeros((B, 1, H, W), np.int32)
    for c in range(NCORES):
        b, q = divmod(c, B * 2)
        out[b, 0, RPC * q: RPC * (q + 1), :] = results[c]["y"]
        out[b, 0, H - SR:, SC * q: SC * (q + 1)] = results[c]["ys"]
    return out


def kernel(gtmasks):
    global LAST_EXEC_NS, LAST_RESULTS
    in_maps = make_in_maps(gtmasks)
    nc = build_nc()
    res = bass_utils.run_bass_kernel_spmd(
        nc, in_maps, core_ids=list(range(NCORES)), trace=PROFILE)
    LAST_EXEC_NS = res.exec_time_ns
    LAST_RESULTS = res
    return assemble(res.results)

